# revision 31
# baseline (speedup 1.0000x reference)
"""Trainium2 Bass kernel for nn_Attention_20933670601301.

Math (per batch b, with P[b] in [n, C], n=512, C=256):
    p_sel = P[b, id[b]]                       # [C]   (gathered host-side)
    qk    = Wk^T (Wq p_sel + bq) = M p_sel + v    (M, v folded on host)
    scores= P[b] @ qk  (+ const)              # [n]; const cancels in softmax
    attn  = softmax(scores)
    out   = Wv @ (P[b]^T attn) + bv

Layout strategy: the host ships P TRANSPOSED per batch (PT[b] = P[b]^T,
fp16, 8 MiB/core -- the only big stream; the DMA is the roofline at
~24 us).  Both big contractions run on the PE as stationary-weight
matmuls whose moving side is a single column (cost ~0):
  scores column: lhsT = PT chunk [c-part, n-cols], rhs = qk col  -> [n,1]
  t^T    column: lhsT = P  chunk [n-part, c-cols], rhs = attn col -> [c,1]
The natural-layout P needed by the t-stage is regenerated on-chip with
PE transposes (fp16 -> fp16 PSUM, 1 bank/batch) whose PSUM->SBUF copies
are split between DVE (2x mode) and ACT, balanced against their other
work; GPSIMD cannot read PSUM.

Softmax runs on 2 groups of 16 batches (engine cost per group is
free-size-bound, so fewer groups = less work): PE transposes scores to
[b, n], DVE negated-max, ACT exp with accumulated sum, and the
transpose-back matmul uses diag(1/esum) as its stationary so attnT
lands already normalized.  bv rides the output matmul as a rank-1
(bv-row x ones) accumulation term, and the output leaves in the
PE-native transposed layout [mc, c, b] which the host untransposes.

Emission order software-pipelines everything against the DMA stream
(in-order queues: stages first so a finished group's PSUM bank frees
before the next claims it; out-DMAs ride Pool/late-SP so they never
block input loads at a queue head; the last pair's conversions are
emitted after the final softmax head).

Data-parallel across 8 cores on the batch dim; weights replicated,
fused + pre-transposed on the host to match the PE's lhsT layout.
"""

import numpy as np

B, N, C = 256, 512, 256
NCORES = 8
BL = B // NCORES      # 32 batches per core
NK = N // 128         # 4 chunks of 128 rows
G = 16                # softmax group size (batches): engine cost per group
                      # is free-size-bound (independent of G), so fewer,
                      # larger groups mean less total softmax work
NG = BL // G

_CACHE = {}


def _build():
    from contextlib import ExitStack

    import concourse.bass as bass
    import concourse.mybir as mybir
    import concourse.tile as tile
    from concourse import bacc
    from concourse.masks import make_identity

    dt = mybir.dt
    AF = mybir.ActivationFunctionType
    OP = mybir.AluOpType
    f32, f16 = dt.float32, dt.float16

    nc = bacc.Bacc("TRN2", target_bir_lowering=False)
    pt_d = nc.dram_tensor("pt", [BL, C, N], f16, kind="ExternalInput")
    # p_sel = P[b, id[b]] is gathered host-side: the device indirect-DMA path
    # (DynamicDMA) is disabled in this walrus build and hard-faults the NRT
    psel_d = nc.dram_tensor("psel", [BL, C], f32, kind="ExternalInput")
    # packed host-side in final SBUF layout: w[kp, wi, kc, mc, mp]
    # with w0 = M^T, w1 = Wv^T  (M = Wk^T Wq);  b = [v, bv]
    w_d = nc.dram_tensor("w", [128, 2, 2, 2, 128], f16, kind="ExternalInput")
    b_d = nc.dram_tensor("b", [2, C], f32, kind="ExternalInput")
    bvr_d = nc.dram_tensor("bvr", [2, 128], f16, kind="ExternalInput")
    # output stays in the PE-native transposed layout [mc, c-part, b];
    # the host untransposes it for free
    out_d = nc.dram_tensor("out", [2, 128, BL], f32, kind="ExternalOutput")

    with tile.TileContext(nc) as tc, ExitStack() as ctx:
        consts = ctx.enter_context(tc.tile_pool(name="consts", bufs=1))
        big = ctx.enter_context(tc.tile_pool(name="big", bufs=1))
        sgrp = ctx.enter_context(tc.tile_pool(name="sgrp", bufs=2))
        # PSUM banks: ptp 4 + scg 1 + ptt 1 + psm 2 = 8 of 8
        # (pools allocate bufs slots per tag, bank-granular)
        ptp = ctx.enter_context(tc.tile_pool(name="ptp", bufs=4, space="PSUM"))
        scT = ctx.enter_context(tc.tile_pool(name="scT", bufs=1, space="PSUM"))
        ptt = ctx.enter_context(tc.tile_pool(name="ptt", bufs=1, space="PSUM"))
        psm = ctx.enter_context(tc.tile_pool(name="psm", bufs=2, space="PSUM"))

        # ---- early DMAs first: nothing may delay the pt stream ----
        p_sel = consts.tile([BL, C], f32)

        # ---- persistent state ----
        pt_sb = big.tile([128, BL, 2, N], f16)       # PT stream  [c-half part, b, h, n]
        p_nat = big.tile([128, BL, NK, C], f16)      # natural    [n part, b, k, c]
        scores_sb = consts.tile([128, NK, BL], f32)  # [n part, k, b]
        attnT = consts.tile([128, NK, BL], f16)
        tT_sb = consts.tile([128, 2, BL], f16)
        w_sb = consts.tile([128, 2, 2, 2, 128], f16)
        b_sb = consts.tile([128, 2, 2], f32)
        wm_sb = w_sb[:, 0]
        wvt_sb = w_sb[:, 1]
        v_sb = b_sb[:, 0]
        qkT_sb = consts.tile([128, 2, BL], f32)
        qk16 = consts.tile([128, 2, BL], f16)
        outT_sb = consts.tile([128, 2, BL], f32)
        bvr_sb = consts.tile([1, 2, 128], f16)
        ones16 = consts.tile([1, BL], f16)

        def load_weights():
            nc.scalar.dma_start(out=w_sb, in_=w_d[:, :, :, :, :])
            nc.scalar.dma_start(
                out=b_sb, in_=b_d[:, :].rearrange("w (c p) -> p w c", p=128)
            )
            nc.scalar.dma_start(out=bvr_sb, in_=bvr_d[None, :, :])

        def setup_qk():
            # p_selT [C-part, b]
            p_selT = consts.tile([128, 2, BL], f16)
            for h in range(2):
                pst = psm.tile([128, BL], f32, tag="s")
                nc.tensor.transpose(
                    out=pst,
                    in_=p_sel[:, h * 128 : (h + 1) * 128],
                    identity=ident[:BL, :BL],
                )
                nc.vector.tensor_copy(out=p_selT[:, h, :], in_=pst)
            # qk^T = M @ p_sel^T + v  -> [C-part, b]
            qk_ps = psm.tile([128, 2, BL], f32, tag="s")
            for mc in range(2):
                for kc in range(2):
                    nc.tensor.matmul(
                        out=qk_ps[:, mc, :],
                        lhsT=wm_sb[:, kc, mc, :],
                        rhs=p_selT[:, kc, :],
                        start=(kc == 0),
                        stop=(kc == 1),
                    )
            for mc in range(2):
                nc.scalar.activation(
                    out=qkT_sb[:, mc, :],
                    in_=qk_ps[:, mc, :],
                    func=AF.Identity,
                    bias=v_sb[:, mc : mc + 1],
                    scale=1.0,
                )
            nc.vector.tensor_copy(out=qk16, in_=qkT_sb)

        def load_part(b0, nb):
            nc.sync.dma_start(
                out=pt_sb[:, b0 : b0 + nb],
                in_=pt_d[b0 : b0 + nb, :, :].rearrange(
                    "b (h p) n -> p b h n", p=128
                ),
            )

        # scores PSUM group tiles (accumulated across the group's batches)
        sc_ps = {}

        def scores_batch(b):
            g, j = divmod(b, G)
            if j == 0:
                sc_ps[g] = scT.tile([128, NK, G], f32, tag="scg", name="scg")
            t = sc_ps[g]
            for k in range(NK):
                for h in range(2):
                    nc.tensor.matmul(
                        out=t[:, k, j : j + 1],
                        lhsT=pt_sb[:, b, h, k * 128 : (k + 1) * 128],
                        rhs=qk16[:, h, b : b + 1],
                        start=(h == 0),
                        stop=(h == 1),
                    )

        # PSUM->SBUF copy engine per batch: GPSIMD cannot read PSUM, so the
        # copies split between DVE and ACT, balanced against their other work
        cp_eng = []
        acc = {"D": 0.0, "A": 0.0}
        rate = {"D": 0.66, "A": 1.04}  # us per copy
        base = {"D": 2.4, "A": 3.6}    # other busy work
        for _ in range(BL):
            e = min(acc, key=lambda k: base[k] + acc[k] + rate[k])
            acc[e] += rate[e]
            cp_eng.append(e)
        # the chain engines (ACT above all) must be clear of conversion
        # copies when the final group's softmax head runs
        cp_eng[BL - 6 :] = ["D", "D", "A", "A", "D", "D"]

        def trans_batch(b):
            tp = ptp.tile([128, NK, 2, 128], f16, tag="ptp")
            for k in range(NK):
                for h in range(2):
                    nc.tensor.transpose(
                        out=tp[:, k, h, :],
                        in_=pt_sb[:, b, h, k * 128 : (k + 1) * 128],
                        identity=ident16,
                    )
            e = cp_eng[b]
            dst = p_nat[:, b, :, :].rearrange("p k (h c) -> p k h c", h=2)
            if e == "D":
                nc.vector.tensor_copy(out=dst, in_=tp)
            else:
                nc.scalar.copy(out=dst, in_=tp)

        # ---- softmax + t + out stages (per group) ----
        grp_state = {}

        def phase_b1(g):
            gs = slice(g * G, (g + 1) * G)
            nc.vector.tensor_copy(out=scores_sb[:, :, gs], in_=sc_ps.pop(g))
            sp = psm.tile([G, NK, 128], f32, tag="s")
            for k in range(NK):
                nc.tensor.transpose(
                    out=sp[:, k, :],
                    in_=scores_sb[:, k, gs],
                    identity=ident,
                )
            grp_state[g] = sp

        def phase_b2a(g):
            # max/exp read the transposed scores straight from PSUM
            sc_nat = grp_state[g]
            negmax = sgrp.tile([G, 1], f32, tag="negmax")
            nc.vector.tensor_reduce(
                out=negmax,
                in_=sc_nat[:, :, :],
                axis=mybir.AxisListType.XY,
                op=OP.max,
                negate=True,
            )
            grp_state[g] = (sc_nat, negmax)

        def phase_b2b(g):
            sc_nat, negmax = grp_state[g]
            attn_nat = sgrp.tile([G, N], f32, tag="attnnat")
            esum = sgrp.tile([G, 1], f32, tag="esum")
            nc.scalar.activation(
                out=attn_nat,
                in_=sc_nat[:, :, :],
                func=AF.Exp,
                bias=negmax[:, :1],
                scale=1.0,
                accum_out=esum,
            )
            grp_state[g] = (attn_nat, esum)

        def phase_b3(g):
            # the transpose-back doubles as the softmax normalize: its
            # "identity" is diag(1/esum), so attnT lands already normalized
            attn_nat, esum = grp_state.pop(g)
            rs = sgrp.tile([G, 1], f32, tag="rs")
            nc.vector.reciprocal(rs, esum)
            dtile = sgrp.tile([G, G], f32, tag="dtile")
            nc.vector.tensor_scalar_mul(dtile, ident[:G, :G], rs[:, :1])
            ap_ps = psm.tile([128, NK, G], f32, tag="s")
            for k in range(NK):
                nc.tensor.matmul(
                    out=ap_ps[:, k, :],
                    lhsT=attn_nat[:, k * 128 : (k + 1) * 128],
                    rhs=dtile,
                )
            nc.vector.tensor_copy(
                out=attnT[:, :, g * G : (g + 1) * G], in_=ap_ps
            )

        def phase_t(g):
            gs = slice(g * G, (g + 1) * G)
            tT_g = ptt.tile([128, 2, G], f32, tag="tTg")
            for j in range(G):
                b = g * G + j
                for h in range(2):
                    for k in range(NK):
                        nc.tensor.matmul(
                            out=tT_g[:, h, j : j + 1],
                            lhsT=p_nat[:, b, k, h * 128 : (h + 1) * 128],
                            rhs=attnT[:, k, b : b + 1],
                            start=(k == 0),
                            stop=(k == NK - 1),
                        )
            nc.vector.tensor_copy(out=tT_sb[:, :, gs], in_=tT_g)

        def phase_d(g, eng=None):
            gs = slice(g * G, (g + 1) * G)
            o_ps = psm.tile([128, 2, G], f32, tag="s")
            for mc in range(2):
                # bv folded into the matmul as a rank-1 (bv-row x ones) term
                for kc in range(2):
                    nc.tensor.matmul(
                        out=o_ps[:, mc, :],
                        lhsT=wvt_sb[:, kc, mc, :],
                        rhs=tT_sb[:, kc, gs],
                        start=(kc == 0),
                        stop=False,
                    )
                nc.tensor.matmul(
                    out=o_ps[:, mc, :],
                    lhsT=bvr_sb[:, mc, :],
                    rhs=ones16[:, gs],
                    start=False,
                    stop=True,
                )
            # the two bias adds run in parallel (ACT / DVE) and each c-half
            # DMAs out on its own path; mid-run outs ride the idle Pool SWDGE
            # (SP/ACT queues would block later input loads), the last group
            # takes SP + Pool, both empty by then
            dma_eng = eng if eng is not None else nc.gpsimd
            nc.scalar.copy(out=outT_sb[:, :, gs], in_=o_ps)
            dma_eng.dma_start(
                out=out_d[:, :, gs].rearrange("m p g -> p m g"),
                in_=outT_sb[:, :, gs],
            )

        # ---- schedule ----
        chunks = [(0, 1), (1, 1)] + [(b0, 2) for b0 in range(2, BL, 2)]
        stages = [
            (phase_b1, 0),
            (phase_b2a, 2),
            (phase_b2b, 4),
            (lambda g: (phase_b3(g), phase_t(g)), 6),
            (phase_d, 6),
        ]
        nstage = [0] * len(stages)

        def run_stages(done_a):
            for si, (fn, off) in enumerate(stages):
                lim = nstage[si - 1] if si else NG
                while nstage[si] < lim and done_a >= nstage[si] * G + G + off:
                    fn(nstage[si])
                    nstage[si] += 1

        # first two single-batch loads are emitted before the weight DMAs;
        # their compute is emitted only AFTER setup_qk has written qk16
        # (Tile tracks dependencies in emission order).
        for b0, nb in chunks[:2]:
            load_part(b0, nb)
        # SWDGE (Pool) keeps the HWDGE slot free for the pt pairs
        nc.gpsimd.dma_start(out=p_sel, in_=psel_d[:, :])
        ident = consts.tile([128, 128], f32)
        make_identity(nc, ident)
        ident16 = consts.tile([128, 128], f16)
        make_identity(nc, ident16)
        nc.gpsimd.memset(ones16, 1.0)
        load_weights()
        setup_qk()
        for b0, nb in chunks[:2]:
            for b in range(b0, b0 + nb):
                scores_batch(b)
                trans_batch(b)
        for b0, nb in chunks[2:-1]:
            load_part(b0, nb)
            # stages first: a finished group's scores PSUM bank is copied out
            # (freeing the single scg slot) before the next group claims it
            run_stages(b0)
            for b in range(b0, b0 + nb):
                scores_batch(b)
                trans_batch(b)
        # last chunk: the final group's softmax head (max, exp) goes ahead
        # of the remaining p_nat conversions in every queue
        b0, nb = chunks[-1]
        load_part(b0, nb)
        run_stages(b0)
        for b in range(b0, b0 + nb):
            scores_batch(b)
        run_stages(BL)

        def flush(si, upto):
            fn = stages[si][0]
            while nstage[si] < upto:
                fn(nstage[si])
                nstage[si] += 1

        flush(0, NG)   # b1(last)
        flush(1, NG)   # max(last)
        flush(2, NG)   # exp(last)
        for b in range(b0, b0 + nb):
            trans_batch(b)
        flush(3, NG)   # b3 + t(last)
        while nstage[4] < NG - 1:
            phase_d(nstage[4])
            nstage[4] += 1
        phase_d(NG - 1, eng=nc.sync)
        nstage[4] += 1

    nc.compile()
    return nc


LAST_RESULT = None


def kernel(P, id, Wq, bq, Wk, bk, Wv, bv):
    global LAST_RESULT
    from concourse.bass_utils import run_bass_kernel_spmd

    P = np.asarray(P, dtype=np.float32)
    idv = np.asarray(id).astype(np.int32)
    Wq = np.asarray(Wq, dtype=np.float32)
    Wk = np.asarray(Wk, dtype=np.float32)
    Wv = np.asarray(Wv, dtype=np.float32)
    bq = np.asarray(bq, dtype=np.float32)
    bv = np.asarray(bv, dtype=np.float32)

    if "nc" not in _CACHE:
        _CACHE["nc"] = _build()
    nc = _CACHE["nc"]

    # fold the Q and K projections into one matrix (host-side weight prep):
    # qk = Wk^T (Wq p + bq) = M p + v;  lhsT layout wants M^T = Wq^T Wk.
    mt = (Wq.T @ Wk).astype(np.float32)
    v = np.ascontiguousarray((Wk.T @ bq).astype(np.float32))
    w = np.ascontiguousarray(
        np.stack([mt, Wv.T])
        .reshape(2, 2, 128, 2, 128)
        .transpose(2, 0, 1, 3, 4)
        .astype(np.float16)
    )
    bb = np.ascontiguousarray(np.stack([v, bv]))

    in_maps = []
    for c in range(NCORES):
        sl = slice(c * BL, (c + 1) * BL)
        Pc = P[sl]
        in_maps.append(
            {
                "pt": np.ascontiguousarray(
                    Pc.transpose(0, 2, 1).astype(np.float16)
                ),
                "psel": np.ascontiguousarray(Pc[np.arange(BL), idv[sl]]),
                "w": w,
                "b": bb,
                "bvr": np.ascontiguousarray(
                    bv.reshape(2, 128).astype(np.float16)
                ),
            }
        )

    res = run_bass_kernel_spmd(nc, in_maps, core_ids=list(range(NCORES)))
    LAST_RESULT = res
    out = np.concatenate(
        [r["out"].transpose(2, 0, 1).reshape(BL, C) for r in res.results], axis=0
    )
    return out
